# revision 2
# baseline (speedup 1.0000x reference)
"""CenterLoss on Trainium2 (Bass/Tile), 8 NeuronCores.

loss = sum_b ||x[b] - centers[labels[b]]||^2 / B
with B=1024, D=512, C=100000 classes.

Strategy (vocab-parallel, per the sharding hint): centers are sharded
row-wise across the 8 cores (12500 rows each). The host routes each batch
row to the core that owns its label's shard (this is the sharding step —
pure data routing). Each core then indirect-DMA-gathers its assigned
center rows out of its own shard, computes per-row squared distances
(two DVE ops per 128-row tile), and returns per-row partial sums. The
host sums the real (non-pad) entries and divides by B.
"""

import numpy as np

B = 1024
D = 512
C = 100000
M = 8  # cores
S = C // M  # 12500 center rows per shard
P = 128  # SBUF partitions

_cache: dict = {}
last_results = None


def _build(T: int):
    import concourse.bacc as bacc
    import concourse.bass as bass
    import concourse.tile as tile
    from concourse import mybir

    nc = bacc.Bacc("TRN2", target_bir_lowering=False, debug=False)
    f32 = mybir.dt.float32
    i32 = mybir.dt.int32

    xs = nc.dram_tensor("xs", [T, P, D], f32, kind="ExternalInput")
    idx = nc.dram_tensor("idx", [T, P, 1], i32, kind="ExternalInput")
    cs = nc.dram_tensor("cs", [S, D], f32, kind="ExternalInput")
    out = nc.dram_tensor("out", [P, T], f32, kind="ExternalOutput")

    with tile.TileContext(nc) as tc:
        with (
            tc.tile_pool(name="work", bufs=2) as pool,
            tc.tile_pool(name="res", bufs=1) as rpool,
        ):
            partial = rpool.tile([P, T], f32)
            for t in range(T):
                idx_t = pool.tile([P, 1], i32)
                nc.sync.dma_start(out=idx_t[:], in_=idx[t])
                x_t = pool.tile([P, D], f32)
                nc.sync.dma_start(out=x_t[:], in_=xs[t])
                c_t = pool.tile([P, D], f32)
                nc.gpsimd.indirect_dma_start(
                    out=c_t[:],
                    out_offset=None,
                    in_=cs[:],
                    in_offset=bass.IndirectOffsetOnAxis(ap=idx_t[:, :1], axis=0),
                )
                d_t = pool.tile([P, D], f32)
                nc.vector.tensor_tensor(
                    out=d_t[:], in0=x_t[:], in1=c_t[:], op=mybir.AluOpType.subtract
                )
                sq_t = pool.tile([P, D], f32)
                nc.scalar.activation(
                    out=sq_t[:],
                    in_=d_t[:],
                    func=mybir.ActivationFunctionType.Square,
                    accum_out=partial[:, t : t + 1],
                )
            nc.sync.dma_start(out=out[:], in_=partial[:])
    nc.compile()
    return nc


def _shard_inputs(x, labels, centers):
    """Host-side routing: batch rows -> owning core, padded to T*128 rows."""
    shard = labels // S
    local = (labels % S).astype(np.int32)
    rows = [np.nonzero(shard == c)[0] for c in range(M)]
    maxn = max(len(r) for r in rows)
    T = max(1, -(-maxn // P))
    in_maps = []
    for c in range(M):
        r = rows[c]
        xs = np.zeros((T * P, D), dtype=np.float32)
        xs[: len(r)] = x[r]
        idxs = np.zeros((T * P, 1), dtype=np.int32)
        idxs[: len(r), 0] = local[r]
        in_maps.append(
            {
                "xs": xs.reshape(T, P, D),
                "idx": idxs.reshape(T, P, 1),
                "cs": centers[c * S : (c + 1) * S],
            }
        )
    return T, rows, in_maps


def kernel(x, labels, centers, _trace=False):
    from concourse.bass_utils import run_bass_kernel_spmd

    x = np.ascontiguousarray(np.asarray(x, dtype=np.float32))
    labels = np.asarray(labels).astype(np.int64)
    centers = np.ascontiguousarray(np.asarray(centers, dtype=np.float32))

    T, rows, in_maps = _shard_inputs(x, labels, centers)

    if T not in _cache:
        _cache[T] = _build(T)
    nc = _cache[T]

    res = run_bass_kernel_spmd(nc, in_maps, core_ids=list(range(M)), trace=_trace)
    global last_results
    last_results = res

    total = 0.0
    for c in range(M):
        part = np.asarray(res.results[c]["out"])  # [P, T]; entry j -> [j % P, j // P]
        n = len(rows[c])
        total += part.T.reshape(-1)[:n].sum(dtype=np.float64)
    return np.asarray(total / B, dtype=np.float32)


# revision 3
# speedup vs baseline: 1.0996x; 1.0996x over previous
"""CenterLoss on Trainium2 (Bass, raw engine programming), 8 NeuronCores.

loss = sum_b ||x[b] - centers[labels[b]]||^2 / B
with B=1024, D=512, C=100000 classes (hardcoded below).

Sharding (class/vocab-parallel, load-balanced): sort the batch by label and
give each of the 8 cores exactly 128 rows. Each core's labels then span a
contiguous class range, so it receives a contiguous row-slice of `centers`
(a zero-copy numpy view) plus shard-local indices. On the device each core:

  1. DMAs its 128 local indices and 128 x rows into SBUF,
  2. indirect-DMA-gathers its 128 center rows out of its centers slice,
  3. computes d = x - c (DVE) and per-row sum d^2 (DVE fused square+accum),
  4. reduces the 128 per-partition partial sums to one scalar with a
     ones-vector matmul on the tensor engine (partition-dim reduction),
  5. DMAs the 4-byte scalar partial loss out.

The host sums the 8 scalar partials (float64) and divides by B.

Raw Bass (no TileContext) keeps the kernel at ~20 instructions with
hand-placed semaphores; the Tile framework's prologue/epilogue barriers
were measured to cost several microseconds on this 19 us kernel.
"""

import numpy as np

B = 1024
D = 512
C = 100000
M = 8  # cores
P = 128  # SBUF partitions = rows per core (B == M * P)

_cache: dict = {}
last_results = None


def _build(W: int):
    import concourse.bass as bass
    from concourse import mybir

    nc = bass.Bass(
        "TRN2", target_bir_lowering=False, debug=False, enable_partition_id=False
    )
    f32, i32 = mybir.dt.float32, mybir.dt.int32

    xs = nc.dram_tensor("xs", [P, D], f32, kind="ExternalInput")
    idx = nc.dram_tensor("idx", [P, 1], i32, kind="ExternalInput")
    cs = nc.dram_tensor("cs", [W, D], f32, kind="ExternalInput")
    ones = nc.dram_tensor("ones", [P, 1], f32, kind="ExternalInput")
    out = nc.dram_tensor("out", [1, 1], f32, kind="ExternalOutput")

    with (
        nc.sbuf_tensor([P, 1], i32) as idx_sb,
        nc.sbuf_tensor([P, D], f32) as x_sb,
        nc.sbuf_tensor([P, D], f32) as c_sb,
        nc.sbuf_tensor([P, D], f32) as d_sb,
        nc.sbuf_tensor([P, D], f32) as sq_sb,
        nc.sbuf_tensor([P, 1], f32) as ones_sb,
        nc.sbuf_tensor([P, 1], f32) as acc_sb,
        nc.sbuf_tensor([1, 1], f32) as fin_sb,
        nc.psum_tensor([1, 1], f32) as psum_t,
        nc.semaphore() as s_i,
        nc.semaphore() as s_x,
        nc.semaphore() as s_o,
        nc.semaphore() as s_c,
        nc.semaphore() as s_acc,
        nc.semaphore() as s_mm,
        nc.semaphore() as s_fin,
        nc.semaphore() as s_out,
        nc.Block() as block,
    ):

        @block.sync
        def _(sync):
            sync.dma_start(out=idx_sb[:], in_=idx[:]).then_inc(s_i, 16)
            sync.dma_start(out=x_sb[:], in_=xs[:]).then_inc(s_x, 16)
            sync.dma_start(out=ones_sb[:], in_=ones[:]).then_inc(s_o, 16)
            sync.wait_ge(s_fin, 1)
            # completion is covered by the end-of-block DMA drain
            sync.dma_start(out=out[:], in_=fin_sb[:1, :1]).then_inc(s_out, 16)

        @block.gpsimd
        def _(gpsimd):
            gpsimd.wait_ge(s_i, 16)
            gpsimd.indirect_dma_start(
                out=c_sb[:],
                out_offset=None,
                in_=cs[:],
                in_offset=bass.IndirectOffsetOnAxis(ap=idx_sb[:, :1], axis=0),
            ).then_inc(s_c, 16)

        @block.vector
        def _(vector):
            vector.wait_ge(s_c, 16)
            vector.wait_ge(s_x, 16)
            vector.tensor_tensor(
                out=d_sb[:], in0=x_sb[:], in1=c_sb[:], op=mybir.AluOpType.subtract
            )
            # same-engine RAW on d_sb: drain the DVE pipeline
            vector.drain()
            vector.scalar_tensor_tensor(
                out=sq_sb[:],
                in0=d_sb[:],
                scalar=1.0,
                in1=d_sb[:],
                op0=mybir.AluOpType.mult,
                op1=mybir.AluOpType.mult,
                accum_out=acc_sb[:],
            ).then_inc(s_acc, 1)
            vector.wait_ge(s_mm, 1)
            vector.tensor_copy(out=fin_sb[:1, :1], in_=psum_t[:1, :1]).then_inc(
                s_fin, 1
            )

        @block.tensor
        def _(tensor):
            tensor.wait_ge(s_o, 16)
            tensor.wait_ge(s_acc, 1)
            tensor.matmul(
                psum_t[:1, :1],
                ones_sb[:, :1],
                acc_sb[:, :1],
                start=True,
                stop=True,
            ).then_inc(s_mm, 1)

    return nc


def _shard_inputs(x, labels, centers):
    """Sort batch rows by label; 128 rows per core, contiguous class range."""
    order = np.argsort(labels, kind="stable")
    groups = order.reshape(M, P)
    lo = np.array([labels[g[0]] for g in groups])
    hi = np.array([labels[g[-1]] for g in groups])
    W = int(max(2048, -(-int((hi - lo + 1).max()) // 4096) * 4096))
    W = min(W, C)
    lo = np.minimum(lo, C - W)
    ones = np.ones((P, 1), dtype=np.float32)
    in_maps = []
    for c in range(M):
        g = groups[c]
        in_maps.append(
            {
                "xs": np.ascontiguousarray(x[g]),
                "idx": (labels[g] - lo[c]).astype(np.int32).reshape(P, 1),
                "cs": centers[lo[c] : lo[c] + W],
                "ones": ones,
            }
        )
    return W, in_maps


def kernel(x, labels, centers, _trace=False):
    from concourse.bass_utils import run_bass_kernel_spmd

    x = np.ascontiguousarray(np.asarray(x, dtype=np.float32))
    labels = np.asarray(labels).astype(np.int64)
    centers = np.ascontiguousarray(np.asarray(centers, dtype=np.float32))

    W, in_maps = _shard_inputs(x, labels, centers)

    if W not in _cache:
        _cache[W] = _build(W)
    nc = _cache[W]

    res = run_bass_kernel_spmd(nc, in_maps, core_ids=list(range(M)), trace=_trace)
    global last_results
    last_results = res

    total = sum(float(res.results[c]["out"][0, 0]) for c in range(M))
    return np.asarray(total / B, dtype=np.float32)


# revision 4
# speedup vs baseline: 1.1781x; 1.0714x over previous
"""CenterLoss on Trainium2 (Bass, raw engine programming), 8 NeuronCores.

loss = sum_b ||x[b] - centers[labels[b]]||^2 / B
with B=1024, D=512, C=100000 classes (hardcoded below).

Sharding (class/vocab-parallel, load-balanced): sort the batch by label and
give each of the 8 cores exactly 128 rows. Each core's labels then span a
contiguous class range, so it receives a contiguous row-slice of `centers`
(a zero-copy numpy view) plus shard-local indices. On the device each core:

  1. DMAs its 128 local indices and 128 x rows into SBUF,
  2. indirect-DMA-gathers its 128 center rows out of its centers slice,
  3. computes d = x - c (DVE) and per-row sum d^2 (DVE fused square+accum),
  4. reduces the 128 per-partition partial sums to one scalar with a
     ones-vector matmul on the tensor engine (partition-dim reduction),
  5. DMAs the 4-byte scalar partial loss out.

The host sums the 8 scalar partials (float64) and divides by B.

Raw Bass (no TileContext) keeps the kernel at ~20 instructions with
hand-placed semaphores; the Tile framework's prologue/epilogue barriers
were measured to cost several microseconds on this ~19 us kernel. The
output completion relies on the end-of-block engine drains, so no engine
stalls waiting on the final 4-byte DMA.
"""

import numpy as np

B = 1024
D = 512
C = 100000
M = 8  # cores
P = 128  # SBUF partitions = rows per core (B == M * P)

_cache: dict = {}
last_results = None


def _build(W: int):
    import concourse.bass as bass
    from concourse import mybir

    nc = bass.Bass(
        "TRN2", target_bir_lowering=False, debug=False, enable_partition_id=False
    )
    f32, i32 = mybir.dt.float32, mybir.dt.int32

    xs = nc.dram_tensor("xs", [P, D], f32, kind="ExternalInput")
    idx = nc.dram_tensor("idx", [P, 1], i32, kind="ExternalInput")
    cs = nc.dram_tensor("cs", [W, D], f32, kind="ExternalInput")
    out = nc.dram_tensor("out", [1, 1], f32, kind="ExternalOutput")

    with (
        nc.sbuf_tensor([P, 1], i32) as idx_sb,
        nc.sbuf_tensor([P, D], f32) as x_sb,
        nc.sbuf_tensor([P, D], f32) as c_sb,
        nc.sbuf_tensor([P, D], f32) as d_sb,
        nc.sbuf_tensor([P, D], f32) as sq_sb,
        nc.sbuf_tensor([P, 1], f32) as ones_sb,
        nc.sbuf_tensor([P, 1], f32) as acc_sb,
        nc.sbuf_tensor([1, 1], f32) as fin_sb,
        nc.psum_tensor([1, 1], f32) as psum_t,
        nc.semaphore() as s_i,
        nc.semaphore() as s_x,
        nc.semaphore() as s_o,
        nc.semaphore() as s_c,
        nc.semaphore() as s_acc,
        nc.semaphore() as s_mm,
        nc.semaphore() as s_fin,
        nc.semaphore() as s_out,
        nc.Block(no_gpsimd_drain=True) as block,
    ):

        @block.sync
        def _(sync):
            sync.dma_start(out=idx_sb[:], in_=idx[:]).then_inc(s_i, 16)
            sync.dma_start(out=x_sb[:], in_=xs[:]).then_inc(s_x, 16)
            sync.wait_ge(s_fin, 1)
            # completion is covered by the end-of-block engine drains
            sync.dma_start(out=out[:], in_=fin_sb[:1, :1]).then_inc(s_out, 16)

        @block.gpsimd
        def _(gpsimd):
            gpsimd.memset(ones_sb[:], 1.0)
            gpsimd.drain()
            gpsimd.sem_inc(s_o, 16)
            gpsimd.wait_ge(s_i, 16)
            gpsimd.indirect_dma_start(
                out=c_sb[:],
                out_offset=None,
                in_=cs[:],
                in_offset=bass.IndirectOffsetOnAxis(ap=idx_sb[:, :1], axis=0),
            ).then_inc(s_c, 16)

        @block.vector
        def _(vector):
            vector.wait_ge(s_c, 16)
            vector.wait_ge(s_x, 16)
            vector.tensor_tensor(
                out=d_sb[:], in0=x_sb[:], in1=c_sb[:], op=mybir.AluOpType.subtract
            )
            # same-engine RAW on d_sb: drain the DVE pipeline
            vector.drain()
            vector.scalar_tensor_tensor(
                out=sq_sb[:],
                in0=d_sb[:],
                scalar=1.0,
                in1=d_sb[:],
                op0=mybir.AluOpType.mult,
                op1=mybir.AluOpType.mult,
                accum_out=acc_sb[:],
            ).then_inc(s_acc, 1)
            vector.wait_ge(s_mm, 1)
            vector.tensor_copy(out=fin_sb[:1, :1], in_=psum_t[:1, :1]).then_inc(
                s_fin, 1
            )

        @block.tensor
        def _(tensor):
            tensor.wait_ge(s_o, 16)
            tensor.wait_ge(s_acc, 1)
            tensor.matmul(
                psum_t[:1, :1],
                ones_sb[:, :1],
                acc_sb[:, :1],
                start=True,
                stop=True,
            ).then_inc(s_mm, 1)

    return nc


def _shard_inputs(x, labels, centers):
    """Sort batch rows by label; 128 rows per core, contiguous class range."""
    order = np.argsort(labels, kind="stable")
    groups = order.reshape(M, P)
    lo = np.array([labels[g[0]] for g in groups])
    hi = np.array([labels[g[-1]] for g in groups])
    W = int(max(2048, -(-int((hi - lo + 1).max()) // 4096) * 4096))
    W = min(W, C)
    lo = np.minimum(lo, C - W)
    in_maps = []
    for c in range(M):
        g = groups[c]
        in_maps.append(
            {
                "xs": np.ascontiguousarray(x[g]),
                "idx": (labels[g] - lo[c]).astype(np.int32).reshape(P, 1),
                "cs": centers[lo[c] : lo[c] + W],
            }
        )
    return W, in_maps


def kernel(x, labels, centers, _trace=False):
    from concourse.bass_utils import run_bass_kernel_spmd

    x = np.ascontiguousarray(np.asarray(x, dtype=np.float32))
    labels = np.asarray(labels).astype(np.int64)
    centers = np.ascontiguousarray(np.asarray(centers, dtype=np.float32))

    W, in_maps = _shard_inputs(x, labels, centers)

    if W not in _cache:
        _cache[W] = _build(W)
    nc = _cache[W]

    res = run_bass_kernel_spmd(nc, in_maps, core_ids=list(range(M)), trace=_trace)
    global last_results
    last_results = res

    total = sum(float(res.results[c]["out"][0, 0]) for c in range(M))
    return np.asarray(total / B, dtype=np.float32)


# revision 5
# speedup vs baseline: 1.2077x; 1.0251x over previous
"""CenterLoss on Trainium2 (Bass, raw engine programming), 8 NeuronCores.

loss = sum_b ||x[b] - centers[labels[b]]||^2 / B
with B=1024, D=512, C=100000 classes (hardcoded below).

Sharding (class/vocab-parallel, load-balanced): sort the batch by label and
give each of the 8 cores exactly 128 rows. Each core's labels then span a
contiguous class range, so it receives a contiguous row-slice of `centers`
(a zero-copy numpy view) plus shard-local indices. On the device each core:

  1. DMAs its 128 local indices and 128 x rows into SBUF,
  2. indirect-DMA-gathers its 128 center rows out of its centers slice,
  3. expands ||x-c||^2 = ||x||^2 - 2 x.c + ||c||^2 per row:
     ||x||^2 runs on the DVE while the gather is still in flight; after the
     gather, x.c (DVE) and ||c||^2 (ACT, table pre-warmed) run in parallel,
     each fused with a per-partition row-sum accumulator,
  4. reduces the three 128-partition accumulators to one scalar with three
     accumulating (+1/-2/+1) ones-vector matmuls on the tensor engine,
  5. DMAs the 4-byte scalar partial loss out.

The host sums the 8 scalar partials (float64) and divides by B.

Raw Bass (no TileContext) keeps the kernel at ~25 instructions with
hand-placed semaphores; the Tile framework's prologue/epilogue barriers
were measured to cost several microseconds on this ~18.5 us kernel. The
output completion relies on the end-of-block engine drains, so no engine
stalls waiting on the final 4-byte DMA.
"""

from contextlib import ExitStack

import numpy as np

B = 1024
D = 512
C = 100000
M = 8  # cores
P = 128  # SBUF partitions = rows per core (B == M * P)

_cache: dict = {}
last_results = None


def _build(W: int):
    import concourse.bass as bass
    from concourse import mybir

    nc = bass.Bass(
        "TRN2", target_bir_lowering=False, debug=False, enable_partition_id=False
    )
    f32, i32 = mybir.dt.float32, mybir.dt.int32

    xs = nc.dram_tensor("xs", [P, D], f32, kind="ExternalInput")
    idx = nc.dram_tensor("idx", [P, 1], i32, kind="ExternalInput")
    cs = nc.dram_tensor("cs", [W, D], f32, kind="ExternalInput")
    out = nc.dram_tensor("out", [1, 1], f32, kind="ExternalOutput")

    es = ExitStack()
    idx_sb = es.enter_context(nc.sbuf_tensor([P, 1], i32))
    x_sb = es.enter_context(nc.sbuf_tensor([P, D], f32))
    c_sb = es.enter_context(nc.sbuf_tensor([P, D], f32))
    xsq_sb = es.enter_context(nc.sbuf_tensor([P, D], f32))
    xc_sb = es.enter_context(nc.sbuf_tensor([P, D], f32))
    csq_sb = es.enter_context(nc.sbuf_tensor([P, D], f32))
    warm_sb = es.enter_context(nc.sbuf_tensor([P, 1], f32))
    ones_sb = es.enter_context(nc.sbuf_tensor([P, 1], f32))
    neg2_sb = es.enter_context(nc.sbuf_tensor([P, 1], f32))
    acc_x = es.enter_context(nc.sbuf_tensor([P, 1], f32))
    acc_xc = es.enter_context(nc.sbuf_tensor([P, 1], f32))
    acc_c = es.enter_context(nc.sbuf_tensor([P, 1], f32))
    fin_sb = es.enter_context(nc.sbuf_tensor([1, 1], f32))
    psum_t = es.enter_context(nc.psum_tensor([1, 1], f32))
    s_i = es.enter_context(nc.semaphore())
    s_x = es.enter_context(nc.semaphore())
    s_o = es.enter_context(nc.semaphore())
    s_c = es.enter_context(nc.semaphore())
    s_ax = es.enter_context(nc.semaphore())
    s_axc = es.enter_context(nc.semaphore())
    s_ac = es.enter_context(nc.semaphore())
    s_mm = es.enter_context(nc.semaphore())
    s_fin = es.enter_context(nc.semaphore())
    s_out = es.enter_context(nc.semaphore())
    with es:
        block = es.enter_context(nc.Block(no_gpsimd_drain=True))

        @block.sync
        def _(sync):
            sync.dma_start(out=idx_sb[:], in_=idx[:]).then_inc(s_i, 16)
            sync.dma_start(out=x_sb[:], in_=xs[:]).then_inc(s_x, 16)
            sync.wait_ge(s_fin, 1)
            # completion is covered by the end-of-block engine drains
            sync.dma_start(out=out[:], in_=fin_sb[:1, :1]).then_inc(s_out, 16)

        @block.gpsimd
        def _(gpsimd):
            gpsimd.memset(ones_sb[:], 1.0)
            gpsimd.memset(neg2_sb[:], -2.0)
            gpsimd.drain()
            gpsimd.sem_inc(s_o, 16)
            gpsimd.wait_ge(s_i, 16)
            gpsimd.indirect_dma_start(
                out=c_sb[:],
                out_offset=None,
                in_=cs[:],
                in_offset=bass.IndirectOffsetOnAxis(ap=idx_sb[:, :1], axis=0),
            ).then_inc(s_c, 16)

        @block.vector
        def _(vector):
            vector.wait_ge(s_x, 16)
            # ||x||^2 row sums while the gather is still in flight
            vector.scalar_tensor_tensor(
                out=xsq_sb[:],
                in0=x_sb[:],
                scalar=1.0,
                in1=x_sb[:],
                op0=mybir.AluOpType.mult,
                op1=mybir.AluOpType.mult,
                accum_out=acc_x[:],
            ).then_inc(s_ax, 1)
            vector.wait_ge(s_c, 16)
            vector.scalar_tensor_tensor(
                out=xc_sb[:],
                in0=x_sb[:],
                scalar=1.0,
                in1=c_sb[:],
                op0=mybir.AluOpType.mult,
                op1=mybir.AluOpType.mult,
                accum_out=acc_xc[:],
            ).then_inc(s_axc, 1)
            vector.wait_ge(s_mm, 1)
            vector.tensor_copy(out=fin_sb[:1, :1], in_=psum_t[:1, :1]).then_inc(
                s_fin, 1
            )

        @block.scalar
        def _(scalar):
            scalar.wait_ge(s_o, 16)
            # warm the ACT Square table during the gather wait
            scalar.activation(
                out=warm_sb[:],
                in_=ones_sb[:, :1],
                func=mybir.ActivationFunctionType.Square,
            )
            scalar.wait_ge(s_c, 16)
            scalar.activation(
                out=csq_sb[:],
                in_=c_sb[:],
                func=mybir.ActivationFunctionType.Square,
                accum_out=acc_c[:],
            ).then_inc(s_ac, 1)

        @block.tensor
        def _(tensor):
            tensor.wait_ge(s_o, 16)
            tensor.wait_ge(s_ax, 1)
            tensor.matmul(
                psum_t[:1, :1],
                ones_sb[:, :1],
                acc_x[:, :1],
                start=True,
                stop=False,
                skip_group_check=True,
            )
            tensor.wait_ge(s_axc, 1)
            tensor.matmul(
                psum_t[:1, :1],
                neg2_sb[:, :1],
                acc_xc[:, :1],
                start=False,
                stop=False,
                skip_group_check=True,
            )
            tensor.wait_ge(s_ac, 1)
            tensor.matmul(
                psum_t[:1, :1],
                ones_sb[:, :1],
                acc_c[:, :1],
                start=False,
                stop=True,
                skip_group_check=True,
            ).then_inc(s_mm, 1)

    return nc


def _shard_inputs(x, labels, centers):
    """Sort batch rows by label; 128 rows per core, contiguous class range."""
    order = np.argsort(labels, kind="stable")
    groups = order.reshape(M, P)
    lo = np.array([labels[g[0]] for g in groups])
    hi = np.array([labels[g[-1]] for g in groups])
    W = int(max(2048, -(-int((hi - lo + 1).max()) // 4096) * 4096))
    W = min(W, C)
    lo = np.minimum(lo, C - W)
    in_maps = []
    for c in range(M):
        g = groups[c]
        in_maps.append(
            {
                "xs": np.ascontiguousarray(x[g]),
                "idx": (labels[g] - lo[c]).astype(np.int32).reshape(P, 1),
                "cs": centers[lo[c] : lo[c] + W],
            }
        )
    return W, in_maps


def kernel(x, labels, centers, _trace=False):
    from concourse.bass_utils import run_bass_kernel_spmd

    x = np.ascontiguousarray(np.asarray(x, dtype=np.float32))
    labels = np.asarray(labels).astype(np.int64)
    centers = np.ascontiguousarray(np.asarray(centers, dtype=np.float32))

    W, in_maps = _shard_inputs(x, labels, centers)

    if W not in _cache:
        _cache[W] = _build(W)
    nc = _cache[W]

    res = run_bass_kernel_spmd(nc, in_maps, core_ids=list(range(M)), trace=_trace)
    global last_results
    last_results = res

    total = sum(float(res.results[c]["out"][0, 0]) for c in range(M))
    return np.asarray(total / B, dtype=np.float32)
